# revision 1
# baseline (speedup 1.0000x reference)
"""DistanceFromAnswerLoss on 8 Trainium2 NeuronCores.

out = 0.1 * sum_{b,c} mask[b,c] * exp(input[b,c])
  mask[b,c] = |c - t_b| / sqrt(sum_c (c - t_b)^2),  mask = 0 where t_b == 0

Sharding: data-parallel over the batch dim (512 rows per core); each core
computes its partial 0.1 * sum, host adds the 8 scalars.

Per-core pipeline (memory-bound target: ~47us DMA floor at 360 GB/s):
  ScalarE : e = exp(x) -> bf16 (one pass, 1.2 GHz, table set exp_and_others)
  VectorE : d = iota - t        (tensor_scalar ptr, 2x/4x packed bf16 mode)
            p = d * e           (tensor_tensor, 2x bf16 mode)
  abs+row-reduce, split to balance engines:
    K_ACT tiles: ScalarE activation(Abs, accum_out) — same table set as exp,
                 so no table reloads; gives sum|p| per partition for free.
    rest:        two scalar_tensor_tensor ops (1x) using
                 |c-t|*e = max(c,t)*e - min(c,t)*e.
The row norm uses the closed form sum_c (c-t)^2 = C*(t-mu)^2 + K computed
once for all 512 rows on [128,4] tiles.  Final 128-partition reduction is a
tiny matmul against a ones vector on the otherwise-idle PE.
"""

import sys
from contextlib import ExitStack

import numpy as np

sys.path.insert(0, "/opt/trn_rl_repo")

import concourse.bass as bass
import concourse.tile as tile
from concourse import bacc, mybir
from concourse.bass_utils import run_bass_kernel_spmd

B = 4096
C = 8192
N_CORES = 8
ROWS = B // N_CORES          # 512 rows per core
RB = ROWS // 128             # 4 row blocks of 128 partitions
W = 4096                     # column tile width (2 MiB DMAs)
NW = C // W
NT = RB * NW                 # 8 big tiles per core
ACT_IDXS = {0, 2, 4, 5, 6}   # tiles whose abs+reduce runs on ScalarE
NSTRIP = 4                   # last tile is split into NSTRIP strips
# others: tensor_reduce(apply_absolute_value) on VectorE
COEFF = 0.1

MU = (C - 1) / 2.0
_S1 = (C - 1) * C // 2
_S2 = (C - 1) * C * (2 * C - 1) // 6
K = float(_S2 - _S1 * _S1 / C)   # sum_c (c-t)^2 = C*(t-MU)^2 + K

F32 = mybir.dt.float32
BF16 = mybir.dt.bfloat16
Af = mybir.ActivationFunctionType
Op = mybir.AluOpType


def _build() -> bass.Bass:
    nc = bacc.Bacc("TRN2", target_bir_lowering=False, debug=False)
    x = nc.declare_dram_parameter("x", [RB, 128, C], F32, isOutput=False)
    # t arrives twice: partition-major [128, RB] for the batched norm chain,
    # and as RB contiguous [128,1] columns for the per-rb ptr scalars
    t = nc.declare_dram_parameter("t", [128, RB], F32, isOutput=False)
    t2 = nc.declare_dram_parameter("t2", [RB, 128, 1], F32, isOutput=False)
    out = nc.declare_dram_parameter("out", [1, 1], F32, isOutput=True)

    with tile.TileContext(nc) as tc, ExitStack() as ctx:
        const_pool = ctx.enter_context(tc.tile_pool(name="const", bufs=1))
        xpool = ctx.enter_context(tc.tile_pool(name="x", bufs=4))
        epool = ctx.enter_context(tc.tile_pool(name="e", bufs=3))
        dpool = ctx.enter_context(tc.tile_pool(name="d", bufs=2))
        ppool = ctx.enter_context(tc.tile_pool(name="p", bufs=2))
        jpool = ctx.enter_context(tc.tile_pool(name="j", bufs=2))
        spool = ctx.enter_context(tc.tile_pool(name="s", bufs=1))
        psum_pool = ctx.enter_context(tc.tile_pool(name="ps", bufs=1, space="PSUM"))

        # --- tiny front matter: t block + norm chain on [128, RB] ---------
        ttile = const_pool.tile([128, RB], F32)
        nc.sync.dma_start(out=ttile[:], in_=t[:, :])
        negmu = const_pool.tile([128, 1], F32)
        nc.vector.memset(negmu[:], -MU)
        ones = const_pool.tile([128, 1], F32)
        nc.vector.memset(ones[:], 1.0)

        # fold COEFF into the norm: 1/sqrt(100*n2) = 0.1/sqrt(n2)
        tsq = spool.tile([128, RB], F32)
        nc.scalar.activation(tsq[:], ttile[:], Af.Square, bias=negmu[:])
        n2 = spool.tile([128, RB], F32)
        nc.vector.tensor_scalar(
            n2[:], tsq[:], float(C) / COEFF**2, K / COEFF**2,
            op0=Op.mult, op1=Op.add,
        )
        norm = spool.tile([128, RB], F32)
        nc.scalar.activation(norm[:], n2[:], Af.Sqrt)
        inv = spool.tile([128, RB], F32)
        nc.vector.reciprocal(inv[:], norm[:])
        nz = spool.tile([128, RB], F32)
        nc.vector.tensor_scalar(nz[:], ttile[:], 0.0, None, op0=Op.not_equal)
        scale = spool.tile([128, RB], F32)
        nc.vector.tensor_tensor(scale[:], inv[:], nz[:], op=Op.mult)
        # contiguous [128,1] per-row-block t scalars — a strided column
        # slice of ttile as the ptr-scalar operand blocks the DVE 4x mode
        tcols = []
        for rb in range(RB):
            tc_rb = const_pool.tile([128, 1], F32, tag=f"tc{rb}")
            nc.sync.dma_start(out=tc_rb[:], in_=t2[rb])
            tcols.append(tc_rb)

        # --- constants: bf16 iota generated in 2048-wide chunks so the
        # first compute tile unblocks early --------------------------------
        iota = const_pool.tile([128, C], BF16)
        CH = 2048
        for ci in range(C // CH):
            nc.gpsimd.iota(
                iota[:, ci * CH:(ci + 1) * CH],
                pattern=[[1, CH]],
                base=ci * CH,
                channel_multiplier=0,
                allow_small_or_imprecise_dtypes=True,
            )

        # --- main loop: dist per row-block, then the 8 [128, W] tiles -----
        # separate per-engine accumulator tiles: a single shared acc tile
        # written by both ACT and DVE breeds conservative cross-engine deps
        accA = const_pool.tile([128, NT], F32)
        accD = const_pool.tile([128, NT], F32)
        nc.vector.memset(accA[:], 0.0)
        nc.vector.memset(accD[:], 0.0)
        accS = const_pool.tile([128, NSTRIP], F32)
        for idx in range(NT - 1):
            rb, cw = divmod(idx, NW)
            xt = xpool.tile([128, W], F32)
            nc.sync.dma_start(out=xt[:], in_=x[rb, :, cw * W:(cw + 1) * W])
            et = epool.tile([128, W], BF16)
            nc.scalar.activation(et[:], xt[:], Af.Exp)
            dt = dpool.tile([128, W], BF16)
            nc.vector.tensor_scalar(
                dt[:], iota[:, cw * W:(cw + 1) * W], tcols[rb][:], None,
                op0=Op.subtract,
            )
            pt = ppool.tile([128, W], BF16)
            nc.vector.tensor_tensor(pt[:], dt[:], et[:], op=Op.mult)
            if idx in ACT_IDXS:
                jt = jpool.tile([128, W], BF16)
                nc.scalar.activation(
                    jt[:], pt[:], Af.Abs, accum_out=accA[:, idx:idx + 1]
                )
            else:
                nc.vector.tensor_reduce(
                    accD[:, idx:idx + 1], pt[:], axis=mybir.AxisListType.X,
                    op=Op.add, apply_absolute_value=True,
                )
        # last tile in NSTRIP strips so the post-DMA tail chain is short
        SW = W // NSTRIP
        rb, cw = divmod(NT - 1, NW)
        for s in range(NSTRIP):
            c0 = cw * W + s * SW
            xs = xpool.tile([128, SW], F32, tag="xs")
            nc.sync.dma_start(out=xs[:], in_=x[rb, :, c0:c0 + SW])
            es = epool.tile([128, SW], BF16, tag="es")
            nc.scalar.activation(es[:], xs[:], Af.Exp)
            ds = dpool.tile([128, SW], BF16, tag="ds")
            nc.vector.tensor_scalar(
                ds[:], iota[:, c0:c0 + SW], tcols[rb][:], None, op0=Op.subtract
            )
            ps = ppool.tile([128, SW], BF16, tag="ps")
            nc.vector.tensor_tensor(ps[:], ds[:], es[:], op=Op.mult)
            nc.vector.tensor_reduce(
                accS[:, s:s + 1], ps[:], axis=mybir.AxisListType.X,
                op=Op.add, apply_absolute_value=True,
            )
        nc.vector.tensor_reduce(
            accD[:, NT - 1:NT], accS[:], axis=mybir.AxisListType.X, op=Op.add
        )

        # --- combine: rowacc[128, RB] -> scaled -> cross-partition sum ----
        # ACT-path tiles are the even idx, DVE-path the odd: per row block
        # rb, cols {rb*NW..} of accA/accD hold its partials (unused cols of
        # each tile are never read)
        accsum = spool.tile([128, NT], F32)
        nc.vector.tensor_add(accsum[:], accA[:], accD[:])
        rowacc = spool.tile([128, RB], F32)
        nc.vector.tensor_reduce(
            rowacc[:], accsum[:].rearrange("p (rb nw) -> p rb nw", nw=NW),
            axis=mybir.AxisListType.X, op=Op.add,
        )
        partials = spool.tile([128, RB], F32)
        nc.vector.tensor_tensor(partials[:], rowacc[:], scale[:], op=Op.mult)
        ptot = psum_pool.tile([1, RB], F32)
        nc.tensor.matmul(ptot[:], ones[:], partials[:], start=True, stop=True)
        tot = spool.tile([1, 1], F32)
        nc.vector.tensor_reduce(
            tot[:], ptot[:], axis=mybir.AxisListType.X, op=Op.add
        )
        nc.sync.dma_start(out=out[:, :], in_=tot[:])

    nc.finalize()
    return nc


_NC = None


def _get_nc() -> bass.Bass:
    global _NC
    if _NC is None:
        _NC = _build()
    return _NC


def make_in_maps(input: np.ndarray, target: np.ndarray) -> list[dict]:
    x = np.ascontiguousarray(np.asarray(input, dtype=np.float32)).reshape(
        N_CORES, RB, 128, C
    )
    # [N_CORES, 128, RB] partition-major targets + [N_CORES, RB, 128, 1]
    t2 = np.ascontiguousarray(
        np.asarray(target).astype(np.float32).reshape(N_CORES, RB, 128, 1)
    )
    t = np.ascontiguousarray(t2[..., 0].transpose(0, 2, 1))
    return [{"x": x[i], "t": t[i], "t2": t2[i]} for i in range(N_CORES)]


def run(input: np.ndarray, target: np.ndarray, trace: bool = False, tmpdir=None):
    nc = _get_nc()
    in_maps = make_in_maps(input, target)
    res = run_bass_kernel_spmd(
        nc, in_maps, list(range(N_CORES)), trace=trace, tmpdir=tmpdir
    )
    total = np.float32(0.0)
    for r in res.results:
        total += np.float32(r["out"].reshape(-1)[0])
    return np.asarray(total, dtype=np.float32), res


def kernel(input: np.ndarray, target: np.ndarray) -> np.ndarray:
    out, _ = run(input, target)
    return out



# revision 8
# speedup vs baseline: 1.0778x; 1.0778x over previous
"""DistanceFromAnswerLoss on 8 Trainium2 NeuronCores.

out = 0.1 * sum_{b,c} mask[b,c] * exp(input[b,c])
  mask[b,c] = |c - t_b| / sqrt(sum_c (c - t_b)^2),  mask = 0 where t_b == 0

Sharding: data-parallel over the batch dim (512 rows per core); each core
emits 128 partial sums, host adds the 8*128 scalars.

Per-core pipeline (memory-bound: ~44us DMA floor at ~390 GB/s):
  ScalarE : e' = exp(x + bias_b) -> bf16, where bias_b = -0.5*ln(100*n2_b)
            folds the 0.1 coefficient, the row L2 norm
            n2_b = C*(t-mu)^2 + K, and the t==0 mask (ln input += 1e35)
            into the mandatory exp pass.  Ln+Exp live in the same
            activation table set (natural_log_exp_and_others): ONE load.
  VectorE : d = |iota - t|   tensor_scalar(subtract, abs_max 0) -> 4x mode
            p = d * e'       tensor_tensor(mult) -> 2x mode
  TensorE : row-reduce via identity-stationary matmuls accumulating
            R[b, j] += p[b, 512k + j] in one PSUM bank; a final DVE
            tensor_reduce collapses R[128,512] -> rs[128,1] -> DMA out.
Tile order is cw-major so the first 4 tiles only need iota[0:4096].
"""

import sys
from contextlib import ExitStack

import numpy as np

sys.path.insert(0, "/opt/trn_rl_repo")

import concourse.bass as bass
import concourse.tile as tile
from concourse import bacc, mybir
from concourse.bass_utils import run_bass_kernel_spmd

B = 4096
C = 8192
N_CORES = 8
ROWS = B // N_CORES          # 512 rows per core
RB = ROWS // 128             # 4 row blocks of 128 partitions
W = 4096                     # column tile width (2 MiB DMAs)
NW = C // W
NT = RB * NW                 # 8 big tiles per core
NSTRIP = 4                   # last tile is split into NSTRIP strips
MMW = 512                    # matmul moving width (= one PSUM bank of f32)
COEFF = 0.1

MU = (C - 1) / 2.0
_S1 = (C - 1) * C // 2
_S2 = (C - 1) * C * (2 * C - 1) // 6
K = float(_S2 - _S1 * _S1 / C)   # sum_c (c-t)^2 = C*(t-MU)^2 + K
# ln input is u = 100*n2*S_LN (+ Z_LN for t==0 rows), kept in [4.6, 1e6]
# so it stays deep inside the HW ln spline table domain.
# bias = -0.5*ln(100*n2) = -0.5*ln(u) - 0.5*ln(1/S_LN)
S_LN = 1e-12
LN_OFF = -0.5 * float(np.log(1.0 / S_LN))
Z_LN = 1e6                       # t==0: bias ~ -20.7, e' ~ 1e-9*e^x (negligible)

F32 = mybir.dt.float32
BF16 = mybir.dt.bfloat16
Af = mybir.ActivationFunctionType
Op = mybir.AluOpType


def _build() -> bass.Bass:
    nc = bacc.Bacc("TRN2", target_bir_lowering=False, debug=False)
    x = nc.declare_dram_parameter("x", [RB, 128, C], F32, isOutput=False)
    # t arrives twice: partition-major [128, RB] for the batched bias chain,
    # and as RB contiguous [128,1] columns for the per-rb ptr scalars
    t = nc.declare_dram_parameter("t", [128, RB], F32, isOutput=False)
    t2 = nc.declare_dram_parameter("t2", [RB, 128, 1], F32, isOutput=False)
    ident = nc.declare_dram_parameter("ident", [128, 128], BF16, isOutput=False)
    out = nc.declare_dram_parameter("out", [128, 1], F32, isOutput=True)

    with tile.TileContext(nc) as tc, ExitStack() as ctx:
        const_pool = ctx.enter_context(tc.tile_pool(name="const", bufs=1))
        xpool = ctx.enter_context(tc.tile_pool(name="x", bufs=4))
        epool = ctx.enter_context(tc.tile_pool(name="e", bufs=3))
        dpool = ctx.enter_context(tc.tile_pool(name="d", bufs=2))
        ppool = ctx.enter_context(tc.tile_pool(name="p", bufs=2))
        spool = ctx.enter_context(tc.tile_pool(name="s", bufs=1))
        psum_pool = ctx.enter_context(tc.tile_pool(name="ps", bufs=1, space="PSUM"))

        # --- first two x-tile DMAs lead the sync queue so the HBM stream
        # starts immediately (tile order is cw-major: (cw, rb)) ------------
        def tile_rc(idx):
            return idx % RB, idx // RB  # rb, cw

        xts = {}
        for idx in range(2):
            rb, cw = tile_rc(idx)
            xt = xpool.tile([128, W], F32)
            nc.sync.dma_start(out=xt[:], in_=x[rb, :, cw * W:(cw + 1) * W])
            xts[idx] = xt

        # --- tiny front matter: t, identity, bias chain -------------------
        ttile = const_pool.tile([128, RB], F32)
        nc.sync.dma_start(out=ttile[:], in_=t[:, :])
        idt = const_pool.tile([128, 128], BF16)
        nc.sync.dma_start(out=idt[:], in_=ident[:, :])
        tcols = []
        for rb in range(RB):
            tc_rb = const_pool.tile([128, 1], F32, tag=f"tc{rb}")
            nc.sync.dma_start(out=tc_rb[:], in_=t2[rb])
            tcols.append(tc_rb)

        # bias_b = -0.5*ln(u) + LN_OFF, u = 100*S_LN*(C*(t-MU)^2+K) + (t==0)*Z_LN
        ts1 = spool.tile([128, RB], F32)
        nc.vector.tensor_scalar(ts1[:], ttile[:], MU, None, op0=Op.subtract)
        tsq = spool.tile([128, RB], F32)
        nc.vector.tensor_tensor(tsq[:], ts1[:], ts1[:], op=Op.mult)
        zm = spool.tile([128, RB], F32)
        nc.vector.tensor_scalar(
            zm[:], ttile[:], 0.0, Z_LN, op0=Op.is_equal, op1=Op.mult
        )
        sc = S_LN / COEFF**2
        n2s = spool.tile([128, RB], F32)
        nc.vector.tensor_scalar(
            n2s[:], tsq[:], float(C) * sc, K * sc, op0=Op.mult, op1=Op.add
        )
        n2z = spool.tile([128, RB], F32)
        nc.vector.tensor_tensor(n2z[:], n2s[:], zm[:], op=Op.add)
        lg = spool.tile([128, RB], F32)
        nc.scalar.activation(lg[:], n2z[:], Af.Ln)
        bias = spool.tile([128, RB], F32)
        nc.vector.tensor_scalar(
            bias[:], lg[:], -0.5, LN_OFF, op0=Op.mult, op1=Op.add
        )
        # contiguous [128,1] bias columns (strided activation-bias operands
        # are a HW risk; copies are ~60ns each)
        bcols = []
        for rb in range(RB):
            bc = spool.tile([128, 1], F32, tag=f"bc{rb}")
            nc.vector.tensor_copy(bc[:], bias[:, rb:rb + 1])
            bcols.append(bc)
        # uint16 sign-clear mask as a ptr scalar (int immediates are a HW
        # encoding risk)
        absmask = spool.tile([128, 1], mybir.dt.uint16)
        nc.vector.memset(absmask[:], 0x7FFF)

        # --- constants: bf16 iota in 2048-wide chunks (first half first) --
        iota = const_pool.tile([128, C], BF16)
        CH = 2048
        for ci in range(C // CH):
            nc.gpsimd.iota(
                iota[:, ci * CH:(ci + 1) * CH],
                pattern=[[1, CH]],
                base=ci * CH,
                channel_multiplier=0,
                allow_small_or_imprecise_dtypes=True,
            )

        # --- main loop ----------------------------------------------------
        R = psum_pool.tile([128, MMW], F32)
        n_mm = (NT - 1) * (W // MMW) + NSTRIP * (W // NSTRIP // MMW)
        mm_i = 0

        def do_tile(xt, rb, c0, width):
            nonlocal mm_i
            et = epool.tile([128, width], BF16, tag="e" if width == W else "es")
            nc.scalar.activation(et[:], xt[:], Af.Exp, bias=bcols[rb][:])
            ds = dpool.tile([128, width], BF16, tag="d0" if width == W else "d0s")
            nc.vector.tensor_scalar(
                ds[:], iota[:, c0:c0 + width], tcols[rb][:], None,
                op0=Op.subtract,
            )
            # |x| for bf16 = clear the sign bit on the uint16 view
            dt = dpool.tile([128, width], BF16, tag="d" if width == W else "dss")
            nc.vector.tensor_scalar(
                dt[:].bitcast(mybir.dt.uint16),
                ds[:].bitcast(mybir.dt.uint16),
                absmask[:], None, op0=Op.bitwise_and,
            )
            pt = ppool.tile([128, width], BF16, tag="p" if width == W else "ps")
            nc.vector.tensor_tensor(pt[:], dt[:], et[:], op=Op.mult)
            for j in range(width // MMW):
                nc.tensor.matmul(
                    R[:], idt[:], pt[:, j * MMW:(j + 1) * MMW],
                    start=(mm_i == 0), stop=(mm_i == n_mm - 1),
                )
                mm_i += 1

        for idx in range(NT - 1):
            rb, cw = tile_rc(idx)
            if idx not in xts:
                xt = xpool.tile([128, W], F32)
                nc.sync.dma_start(out=xt[:], in_=x[rb, :, cw * W:(cw + 1) * W])
            else:
                xt = xts[idx]
            do_tile(xt, rb, cw * W, W)

        # last tile in NSTRIP strips so the post-DMA tail chain is short
        SW = W // NSTRIP
        rb, cw = tile_rc(NT - 1)
        for s in range(NSTRIP):
            c0 = cw * W + s * SW
            xs = xpool.tile([128, SW], F32, tag="xs")
            nc.sync.dma_start(out=xs[:], in_=x[rb, :, c0:c0 + SW])
            do_tile(xs, rb, c0, SW)

        # --- combine: R[128, MMW] -> rs[128,1] -> DRAM --------------------
        rs = spool.tile([128, 1], F32)
        nc.vector.tensor_reduce(
            rs[:], R[:], axis=mybir.AxisListType.X, op=Op.add
        )
        nc.sync.dma_start(out=out[:, :], in_=rs[:])

    nc.finalize()
    return nc


_NC = None


def _get_nc() -> bass.Bass:
    global _NC
    if _NC is None:
        _NC = _build()
    return _NC


def make_in_maps(input: np.ndarray, target: np.ndarray) -> list[dict]:
    x = np.ascontiguousarray(np.asarray(input, dtype=np.float32)).reshape(
        N_CORES, RB, 128, C
    )
    t2 = np.ascontiguousarray(
        np.asarray(target).astype(np.float32).reshape(N_CORES, RB, 128, 1)
    )
    t = np.ascontiguousarray(t2[..., 0].transpose(0, 2, 1))
    ident_bf16 = _to_bf16(np.eye(128, dtype=np.float32))
    return [
        {"x": x[i], "t": t[i], "t2": t2[i], "ident": ident_bf16}
        for i in range(N_CORES)
    ]


def _to_bf16(a: np.ndarray) -> np.ndarray:
    """float32 -> bfloat16 (round-to-nearest-even) as uint16-backed array
    matching what run_bass_kernel_spmd expects for bf16 params."""
    try:
        import ml_dtypes

        return a.astype(ml_dtypes.bfloat16)
    except ImportError:
        u = a.astype(np.float32).view(np.uint32)
        rounded = ((u + 0x7FFF + ((u >> 16) & 1)) >> 16).astype(np.uint16)
        return rounded.view(np.uint16)


def run(input: np.ndarray, target: np.ndarray, trace: bool = False, tmpdir=None):
    nc = _get_nc()
    in_maps = make_in_maps(input, target)
    res = run_bass_kernel_spmd(
        nc, in_maps, list(range(N_CORES)), trace=trace, tmpdir=tmpdir
    )
    total = np.float32(0.0)
    for r in res.results:
        total += np.float32(np.sum(np.asarray(r["out"], dtype=np.float32)))
    return np.asarray(total, dtype=np.float32), res


def kernel(input: np.ndarray, target: np.ndarray) -> np.ndarray:
    out, _ = run(input, target)
    return out
